# revision 8
# baseline (speedup 1.0000x reference)
"""Trainium2 Bass kernel for a pre-LN transformer block (B=4, T=2048, E=512,
H=8, D=64, HID=4096, causal attention scaled by T**-0.5).

Sharding: 8 cores = (batch b = c//2, token-half h = c%2). Token half h owns
the 64-row blocks {j : j % 2 == h} of the sequence — this makes the causal
block schedule identical on every core (local 64-block i attends exactly
k-blocks 0..i), so one SPMD program serves all 8 cores; only the data
(x slices + a [128,64] diagonal mask) differs per core.

Per core: LN1 (full seq for K/V + own rows for Q/residual), QKV projections,
causal attention (scores computed transposed [s, tq]; softmax denominator via
an augmented ones-column in the V matmul; normalization broadcast via a K=1
matmul), Wo projection + residual, LN2, full FFN on own rows.

Matmul operands are bf16 (PE runs bf16 at full rate; fp32 is 4x slower and
does not fit SBUF); all accumulation (PSUM), layernorm, softmax sums and
residual arithmetic stay fp32.
"""

import numpy as np

import concourse.bass as bass
import concourse.tile as tile
from concourse import bacc
from concourse import mybir
from concourse.bass_utils import run_bass_kernel_spmd
from concourse.masks import make_identity

B, T, E = 4, 2048, 512
H, D = 8, 64
HID = H * E
EPS = 1e-5
SCALE = float(T) ** -0.5

F32 = mybir.dt.float32
BF16 = mybir.dt.bfloat16

NT = T // 128          # 16 global token tiles
NTM = 8                # local token tiles (1024 rows)
NE = E // 128          # 4 embed tiles
NG = 4                 # head groups (2 heads each)
NJ = HID // 128        # 32 ffn tiles
LQ = T // 2            # 1024 local q columns
AF = mybir.ActivationFunctionType
ALU = mybir.AluOpType


def _bcast(vec_ap, p, n):
    """AP for DMA-broadcast of a [n] DRAM vector across p partitions."""
    return bass.AP(tensor=vec_ap.tensor, offset=vec_ap.offset,
                   ap=[[0, p]] + list(vec_ap.ap))


def _layernorm(nc, pool, x_in, xn_out, eps_t, g_b, be_b):
    """LN over free dim (512) of a [128, 512] tile: xn = (x-mu)*rstd*g + be."""
    stats = pool.tile([128, 6], F32, tag="ln_stats")
    nc.vector.bn_stats(out=stats[:], in_=x_in)
    mv = pool.tile([128, 2], F32, tag="ln_mv")
    nc.vector.bn_aggr(out=mv[:], in_=stats[:])
    rstd = pool.tile([128, 1], F32, tag="ln_rstd")
    nc.scalar.activation(out=rstd[:], in_=mv[:, 1:2], func=AF.Sqrt,
                         bias=eps_t[:], scale=1.0)
    nc.vector.reciprocal(out=rstd[:], in_=rstd[:])
    nc.vector.tensor_scalar(out=xn_out, in0=x_in, scalar1=mv[:, 0:1],
                            scalar2=rstd[:], op0=ALU.subtract, op1=ALU.mult)
    nc.vector.tensor_mul(xn_out, xn_out, g_b[:])
    nc.vector.tensor_add(xn_out, xn_out, be_b[:])


def build_program():
    nc = bacc.Bacc()
    xf = nc.declare_dram_parameter("x_full", [T, E], F32, isOutput=False)
    xm = nc.declare_dram_parameter("x_mine", [T // 2, E], F32, isOutput=False)
    wq = nc.declare_dram_parameter("wq2d", [E, E], F32, isOutput=False)
    wk = nc.declare_dram_parameter("wk2d", [E, E], F32, isOutput=False)
    wv = nc.declare_dram_parameter("wv2d", [E, E], F32, isOutput=False)
    wo = nc.declare_dram_parameter("wo", [E, E], F32, isOutput=False)
    w1 = nc.declare_dram_parameter("w1", [E, HID], F32, isOutput=False)
    w2 = nc.declare_dram_parameter("w2", [HID, E], F32, isOutput=False)
    b1t = nc.declare_dram_parameter("b1t", [128, NJ], F32, isOutput=False)
    vg1 = nc.declare_dram_parameter("g1", [E], F32, isOutput=False)
    vbe1 = nc.declare_dram_parameter("be1", [E], F32, isOutput=False)
    vg2 = nc.declare_dram_parameter("g2", [E], F32, isOutput=False)
    vbe2 = nc.declare_dram_parameter("be2", [E], F32, isOutput=False)
    vbo = nc.declare_dram_parameter("bo", [E], F32, isOutput=False)
    vb2 = nc.declare_dram_parameter("b2", [E], F32, isOutput=False)
    tri = nc.declare_dram_parameter("tri", [128, 64], F32, isOutput=False)
    out = nc.declare_dram_parameter("out", [T // 2, E], F32, isOutput=True)

    with tile.TileContext(nc) as tc, \
         nc.allow_low_precision(reason="bf16 matmul operands by design"), \
         tc.tile_pool(name="const", bufs=1) as cpool, \
         tc.tile_pool(name="persist", bufs=1) as pp, \
         tc.tile_pool(name="work", bufs=3) as wk_pool, \
         tc.tile_pool(name="ln", bufs=4) as ln_pool, \
         tc.tile_pool(name="ps", bufs=3, space="PSUM") as ps:

        # ---- constants ----
        ident = cpool.tile([128, 128], BF16)
        make_identity(nc, ident)
        tri_sb = cpool.tile([128, 64], BF16)
        nc.gpsimd.dma_start(out=tri_sb[:], in_=tri[:])
        ones1 = cpool.tile([1, 64], BF16)
        nc.vector.memset(ones1[:], 1.0)
        eps_t = cpool.tile([128, 1], F32)
        nc.vector.memset(eps_t[:], EPS)
        b1_sb = cpool.tile([128, NJ], F32)
        nc.sync.dma_start(out=b1_sb[:], in_=b1t[:])
        g1b = cpool.tile([128, E], F32)
        nc.sync.dma_start(out=g1b[:], in_=_bcast(vg1[:], 128, E))
        be1b = cpool.tile([128, E], F32)
        nc.sync.dma_start(out=be1b[:], in_=_bcast(vbe1[:], 128, E))
        g2b = cpool.tile([128, E], F32)
        nc.sync.dma_start(out=g2b[:], in_=_bcast(vg2[:], 128, E))
        be2b = cpool.tile([128, E], F32)
        nc.sync.dma_start(out=be2b[:], in_=_bcast(vbe2[:], 128, E))
        bob = cpool.tile([128, E], F32)
        nc.sync.dma_start(out=bob[:], in_=_bcast(vbo[:], 128, E))
        b2b = cpool.tile([128, E], F32)
        nc.sync.dma_start(out=b2b[:], in_=_bcast(vb2[:], 128, E))

        with tc.tile_pool(name="attn", bufs=1) as apool:
            # projection weights (e-major 2d layout), bf16 [128, NE, 512]
            wq_sb = apool.tile([128, NE, E], BF16)
            wk_sb = apool.tile([128, NE, E], BF16)
            wv_sb = apool.tile([128, NE, E], BF16)
            wo_sb = apool.tile([128, NE, E], BF16)
            for e in range(NE):
                es = slice(e * 128, (e + 1) * 128)
                nc.gpsimd.dma_start(out=wq_sb[:, e, :], in_=wq[es, :])
                nc.gpsimd.dma_start(out=wk_sb[:, e, :], in_=wk[es, :])
                nc.gpsimd.dma_start(out=wv_sb[:, e, :], in_=wv[es, :])
                nc.gpsimd.dma_start(out=wo_sb[:, e, :], in_=wo[es, :])

            xn_mine = apool.tile([128, NTM, E], F32)
            kT_sb = apool.tile([128, NG, T], BF16)
            qT_sb = apool.tile([128, NG, LQ], BF16)
            vaug = apool.tile([128, NT, 8 * 65], BF16)
            oT_sb = apool.tile([128, NG, LQ], BF16)

            # ---- P1: LN1 over full sequence, build xnT_full (bf16) ----
            with tc.tile_pool(name="pA", bufs=1) as pA:
                xnT_full = pA.tile([128, NE, T], BF16)
                for i in range(NT):
                    x_t = wk_pool.tile([128, E], F32, tag="x_in")
                    nc.sync.dma_start(out=x_t[:], in_=xf[i * 128:(i + 1) * 128, :])
                    xn_t = wk_pool.tile([128, E], F32, tag="xn_t")
                    _layernorm(nc, ln_pool, x_t[:], xn_t[:], eps_t, g1b, be1b)
                    xnb = wk_pool.tile([128, E], BF16, tag="xnb")
                    nc.vector.tensor_copy(out=xnb[:], in_=xn_t[:])
                    for e in range(NE):
                        tp = ps.tile([128, 128], BF16, tag="mm")
                        nc.tensor.transpose(tp[:], xnb[:, e * 128:(e + 1) * 128],
                                            ident[:])
                        nc.vector.tensor_copy(
                            out=xnT_full[:, e, i * 128:(i + 1) * 128], in_=tp[:])

                # ---- P2a: K and V projections (full sequence) ----
                for g in range(NG):
                    for c in range(4):
                        cs = slice(c * 512, (c + 1) * 512)
                        kp = ps.tile([128, 512], F32, tag="mm")
                        for e in range(NE):
                            nc.tensor.matmul(
                                kp[:], wk_sb[:, e, g * 128:(g + 1) * 128],
                                xnT_full[:, e, cs],
                                start=(e == 0), stop=(e == NE - 1))
                        nc.vector.tensor_copy(out=kT_sb[:, g, cs], in_=kp[:])
                for i in range(NT):
                    vp = ps.tile([128, 512], F32, tag="mm")
                    for e in range(NE):
                        nc.tensor.matmul(
                            vp[:], xnT_full[:, e, i * 128:(i + 1) * 128],
                            wv_sb[:, e, :], start=(e == 0), stop=(e == NE - 1))
                    for h in range(H):
                        nc.vector.tensor_copy(
                            out=vaug[:, i, h * 65:h * 65 + 64],
                            in_=vp[:, h * 64:(h + 1) * 64])
                    ones_col = vaug[:, i, :].rearrange(
                        "p (h c) -> p h c", c=65)[:, :, 64]
                    nc.vector.memset(ones_col, 1.0)

                # ---- P1b/P2b: LN1 on own rows, transpose, Q projection ----
                xn_mineT = pA.tile([128, NE, LQ], BF16)
                for i in range(NTM):
                    x_t = wk_pool.tile([128, E], F32, tag="x_in")
                    nc.sync.dma_start(out=x_t[:], in_=xm[i * 128:(i + 1) * 128, :])
                    _layernorm(nc, ln_pool, x_t[:], xn_mine[:, i, :],
                               eps_t, g1b, be1b)
                    xnb = wk_pool.tile([128, E], BF16, tag="xnb")
                    nc.vector.tensor_copy(out=xnb[:], in_=xn_mine[:, i, :])
                    for e in range(NE):
                        tp = ps.tile([128, 128], BF16, tag="mm")
                        nc.tensor.transpose(tp[:], xnb[:, e * 128:(e + 1) * 128],
                                            ident[:])
                        nc.vector.tensor_copy(
                            out=xn_mineT[:, e, i * 128:(i + 1) * 128], in_=tp[:])
                for g in range(NG):
                    for c in range(2):
                        cs = slice(c * 512, (c + 1) * 512)
                        qp = ps.tile([128, 512], F32, tag="mm")
                        for e in range(NE):
                            nc.tensor.matmul(
                                qp[:], wq_sb[:, e, g * 128:(g + 1) * 128],
                                xn_mineT[:, e, cs],
                                start=(e == 0), stop=(e == NE - 1))
                        nc.vector.tensor_copy(out=qT_sb[:, g, cs], in_=qp[:])

            # ---- P3: attention per head ----
            ps_oT_cm = tc.tile_pool(name="ps_oT", bufs=1, space="PSUM")
            ps_oT = ps_oT_cm.__enter__()
            for h in range(H):
                g, r0 = h // 2, (h % 2) * 64
                oT_ps = ps_oT.tile([65, LQ], F32, tag="oT")
                for kb in range(NT):
                    c0 = kb * 64
                    chunks = [(c0, 512), (512, 1024)] if c0 < 512 else [(c0, 1024)]
                    for ci, (a, b_) in enumerate(chunks):
                        n = b_ - a
                        sc = ps.tile([128, 512], F32, tag="mm")
                        nc.tensor.matmul(
                            sc[:, :n],
                            kT_sb[r0:r0 + 64, g, kb * 128:(kb + 1) * 128],
                            qT_sb[r0:r0 + 64, g, a:b_],
                            start=True, stop=True)
                        ex = wk_pool.tile([128, 512], BF16, tag="exp")
                        nc.scalar.activation(out=ex[:, :n], in_=sc[:, :n],
                                             func=AF.Exp, scale=SCALE)
                        if ci == 0:
                            nc.vector.tensor_mul(ex[:, 0:64], ex[:, 0:64],
                                                 tri_sb[:])
                        nc.tensor.matmul(
                            oT_ps[:, a:b_],
                            vaug[:, kb, h * 65:(h + 1) * 65],
                            ex[:, :n],
                            start=(kb == 0), stop=(kb == NT - 1),
                            skip_group_check=True)
                se = wk_pool.tile([1, LQ], BF16, tag="se")
                nc.vector.reciprocal(out=se[:], in_=oT_ps[64:65, :])
                nc.vector.tensor_copy(out=oT_sb[r0:r0 + 64, g, :],
                                      in_=oT_ps[0:64, :])
                for c in range(2):
                    cs = slice(c * 512, (c + 1) * 512)
                    rb = ps.tile([64, 512], F32, tag="mm")
                    nc.tensor.matmul(rb[:], ones1[:], se[:, cs],
                                     start=True, stop=True)
                    nc.vector.tensor_mul(oT_sb[r0:r0 + 64, g, cs],
                                         oT_sb[r0:r0 + 64, g, cs], rb[:])

            ps_oT_cm.__exit__(None, None, None)

            # ---- P4: Wo + residual + LN2 + transpose ----
            xn2_sb = pp.tile([128, NTM, E], F32)
            xn2T = pp.tile([128, NE, LQ], BF16)
            for m in range(NTM):
                ms = slice(m * 128, (m + 1) * 128)
                mha = ps.tile([128, 512], F32, tag="mm")
                for g in range(NG):
                    nc.tensor.matmul(mha[:], oT_sb[:, g, ms], wo_sb[:, g, :],
                                     start=(g == 0), stop=(g == NG - 1))
                x2 = wk_pool.tile([128, E], F32, tag="x2")
                nc.vector.tensor_add(x2[:], mha[:], xn_mine[:, m, :])
                nc.vector.tensor_add(x2[:], x2[:], bob[:])
                _layernorm(nc, ln_pool, x2[:], xn2_sb[:, m, :], eps_t, g2b, be2b)
                xnb = wk_pool.tile([128, E], BF16, tag="xnb")
                nc.vector.tensor_copy(out=xnb[:], in_=xn2_sb[:, m, :])
                for e in range(NE):
                    tp = ps.tile([128, 128], BF16, tag="mm")
                    nc.tensor.transpose(tp[:], xnb[:, e * 128:(e + 1) * 128],
                                        ident[:])
                    nc.vector.tensor_copy(
                        out=xn2T[:, e, m * 128:(m + 1) * 128], in_=tp[:])

        # ---- P5: FFN (attention pool freed; W1/W2 held in bf16) ----
        with tc.tile_pool(name="ffn", bufs=1) as fpool:
            w1_sb = fpool.tile([128, NE, HID], BF16)
            for e in range(NE):
                nc.gpsimd.dma_start(out=w1_sb[:, e, :],
                                    in_=w1[e * 128:(e + 1) * 128, :])
            ps_y_cm = tc.tile_pool(name="ps_y", bufs=1, space="PSUM")
            ps_y = ps_y_cm.__enter__()
            w2_sb = fpool.tile([128, NJ, E], BF16)
            nc.gpsimd.dma_start(
                out=w2_sb[:],
                in_=w2.rearrange("(j p) e -> p j e", p=128))
            for c in range(2):
                cs = slice(c * 512, (c + 1) * 512)
                y_ps = [ps_y.tile([128, 512], F32, tag=f"y{tt}",
                                  name=f"y_ps{tt}")
                        for tt in range(4)]
                for j in range(NJ):
                    hp = ps.tile([128, 512], F32, tag="mm")
                    for e in range(NE):
                        nc.tensor.matmul(
                            hp[:], w1_sb[:, e, j * 128:(j + 1) * 128],
                            xn2T[:, e, cs], start=(e == 0), stop=(e == NE - 1))
                    hr = wk_pool.tile([128, 512], BF16, tag="hr")
                    nc.scalar.activation(out=hr[:], in_=hp[:], func=AF.Relu,
                                         bias=b1_sb[:, j:j + 1], scale=1.0)
                    for tt in range(4):
                        nc.tensor.matmul(y_ps[tt][:],
                                         hr[:, tt * 128:(tt + 1) * 128],
                                         w2_sb[:, j, :],
                                         start=(j == 0), stop=(j == NJ - 1))
                for tt in range(4):
                    m = c * 4 + tt
                    yo = wk_pool.tile([128, E], F32, tag="yo")
                    nc.vector.tensor_add(yo[:], y_ps[tt][:], xn2_sb[:, m, :])
                    nc.vector.tensor_add(yo[:], yo[:], b2b[:])
                    nc.sync.dma_start(out=out[m * 128:(m + 1) * 128, :],
                                      in_=yo[:])
            ps_y_cm.__exit__(None, None, None)
    nc.compile()
    return nc


_prog_cache = {}


def _get_program():
    if "nc" not in _prog_cache:
        _prog_cache["nc"] = build_program()
    return _prog_cache["nc"]


def make_in_maps(x, Wq, Wk, Wv, Wo, bo, W1, b1, W2, b2, g1, be1, g2, be2):
    f = lambda a: np.ascontiguousarray(np.asarray(a, dtype=np.float32))
    x = f(x)
    shared = {
        "wq2d": f(np.transpose(np.asarray(Wq), (1, 0, 2)).reshape(E, E)),
        "wk2d": f(np.transpose(np.asarray(Wk), (1, 0, 2)).reshape(E, E)),
        "wv2d": f(np.transpose(np.asarray(Wv), (1, 0, 2)).reshape(E, E)),
        "wo": f(Wo), "w1": f(W1), "w2": f(W2),
        "b1t": f(np.asarray(b1).reshape(NJ, 128).T),
        "g1": f(g1), "be1": f(be1), "g2": f(g2), "be2": f(be2),
        "bo": f(bo), "b2": f(b2),
    }
    in_maps = []
    s_idx = np.arange(128)[:, None]
    p_idx = np.arange(64)[None, :]
    for c in range(8):
        b, h = c // 2, c % 2
        m = dict(shared)
        m["x_full"] = x[b]
        m["x_mine"] = f(x[b].reshape(NT, 2, 64, E)[:, h].reshape(T // 2, E))
        m["tri"] = f((s_idx <= h * 64 + p_idx).astype(np.float32))
        in_maps.append(m)
    return in_maps


def assemble(results):
    out = np.empty((B, T, E), dtype=np.float32)
    for c in range(8):
        b, h = c // 2, c % 2
        out[b].reshape(NT, 2, 64, E)[:, h] = \
            results[c]["out"].reshape(NT, 64, E)
    return out


def kernel(**inputs):
    nc = _get_program()
    in_maps = make_in_maps(**inputs)
    res = run_bass_kernel_spmd(nc, in_maps, core_ids=list(range(8)))
    return assemble(res.results)


# revision 12
# speedup vs baseline: 1.0313x; 1.0313x over previous
"""Trainium2 Bass kernel for a pre-LN transformer block (B=4, T=2048, E=512,
H=8, D=64, HID=4096, causal attention scaled by T**-0.5).

Sharding: 8 cores = (batch b = c//2, token-half h = c%2). Token half h owns
the 64-row blocks {j : j % 2 == h} of the sequence — this makes the causal
block schedule identical on every core (local 64-block i attends exactly
k-blocks 0..i), so one SPMD program serves all 8 cores; only the data
(x slices + a [128,64] diagonal mask) differs per core.

Per core: LN1 (full seq for K/V + own rows for Q/residual), QKV projections,
causal attention (scores computed transposed [s, tq]; softmax denominator via
an augmented ones-column in the V matmul; normalization broadcast via a K=1
matmul), Wo projection + residual, LN2, full FFN on own rows.

Matmul operands are bf16 (PE runs bf16 at full rate; fp32 is 4x slower and
does not fit SBUF); all accumulation (PSUM), layernorm, softmax sums and
residual arithmetic stay fp32.
"""

import numpy as np

import concourse.bass as bass
import concourse.tile as tile
from concourse import bacc
from concourse import mybir
from concourse.bass_utils import run_bass_kernel_spmd
from concourse.masks import make_identity

B, T, E = 4, 2048, 512
H, D = 8, 64
HID = H * E
EPS = 1e-5
SCALE = float(T) ** -0.5

F32 = mybir.dt.float32
BF16 = mybir.dt.bfloat16

NT = T // 128          # 16 global token tiles
NTM = 8                # local token tiles (1024 rows)
NE = E // 128          # 4 embed tiles
NG = 4                 # head groups (2 heads each)
NJ = HID // 128        # 32 ffn tiles
LQ = T // 2            # 1024 local q columns
AF = mybir.ActivationFunctionType
ALU = mybir.AluOpType


def _bcast(vec_ap, p, n):
    """AP for DMA-broadcast of a [n] DRAM vector across p partitions."""
    return bass.AP(tensor=vec_ap.tensor, offset=vec_ap.offset,
                   ap=[[0, p]] + list(vec_ap.ap))


def _layernorm(nc, pool, x_in, xn_out, eps_t, g_b, be_b, tmp_pool=None):
    """LN over free dim (512) of a [128, 512] tile: xn = (x-mu)*rstd*g + be.
    Computed as ((x - mu) * g) * rstd + be via two fused DVE ops."""
    stats = pool.tile([128, 6], F32, tag="ln_stats")
    nc.vector.bn_stats(out=stats[:], in_=x_in)
    mv = pool.tile([128, 2], F32, tag="ln_mv")
    nc.vector.bn_aggr(out=mv[:], in_=stats[:])
    rstd = pool.tile([128, 1], F32, tag="ln_rstd")
    nc.scalar.activation(out=rstd[:], in_=mv[:, 1:2], func=AF.Sqrt,
                         bias=eps_t[:], scale=1.0)
    nc.vector.reciprocal(out=rstd[:], in_=rstd[:])
    tp = (tmp_pool or pool).tile([128, 512], F32, tag="ln_tmp")
    nc.vector.scalar_tensor_tensor(out=tp[:], in0=x_in, scalar=mv[:, 0:1],
                                   in1=g_b[:], op0=ALU.subtract, op1=ALU.mult)
    nc.vector.scalar_tensor_tensor(out=xn_out, in0=tp[:], scalar=rstd[:],
                                   in1=be_b[:], op0=ALU.mult, op1=ALU.add)


def build_program():
    nc = bacc.Bacc()
    xf = nc.declare_dram_parameter("x_full", [T, E], F32, isOutput=False)
    xm = nc.declare_dram_parameter("x_mine", [T // 2, E], F32, isOutput=False)
    wq = nc.declare_dram_parameter("wq2d", [E, E], F32, isOutput=False)
    wk = nc.declare_dram_parameter("wk2d", [E, E], F32, isOutput=False)
    wv = nc.declare_dram_parameter("wv2d", [E, E], F32, isOutput=False)
    wo = nc.declare_dram_parameter("wo", [E, E], F32, isOutput=False)
    w1 = nc.declare_dram_parameter("w1", [E, HID], F32, isOutput=False)
    w2 = nc.declare_dram_parameter("w2", [HID, E], F32, isOutput=False)
    b1t = nc.declare_dram_parameter("b1t", [128, NJ], F32, isOutput=False)
    vg1 = nc.declare_dram_parameter("g1", [E], F32, isOutput=False)
    vbe1 = nc.declare_dram_parameter("be1", [E], F32, isOutput=False)
    vg2 = nc.declare_dram_parameter("g2", [E], F32, isOutput=False)
    vbe2 = nc.declare_dram_parameter("be2", [E], F32, isOutput=False)
    vbo = nc.declare_dram_parameter("bo", [E], F32, isOutput=False)
    vb2 = nc.declare_dram_parameter("b2", [E], F32, isOutput=False)
    tri = nc.declare_dram_parameter("tri", [128, 64], F32, isOutput=False)
    out = nc.declare_dram_parameter("out", [T // 2, E], F32, isOutput=True)

    with tile.TileContext(nc) as tc, \
         nc.allow_low_precision(reason="bf16 matmul operands by design"), \
         tc.tile_pool(name="const", bufs=1) as cpool, \
         tc.tile_pool(name="persist", bufs=1) as pp, \
         tc.tile_pool(name="work", bufs=3) as wk_pool, \
         tc.tile_pool(name="ln", bufs=4) as ln_pool, \
         tc.tile_pool(name="ps", bufs=3, space="PSUM") as ps:

        # ---- constants ----
        ident = cpool.tile([128, 128], BF16)
        make_identity(nc, ident)
        tri_sb = cpool.tile([128, 64], BF16)
        nc.gpsimd.dma_start(out=tri_sb[:], in_=tri[:])
        ones1 = cpool.tile([1, 64], BF16)
        nc.vector.memset(ones1[:], 1.0)
        eps_t = cpool.tile([128, 1], F32)
        nc.vector.memset(eps_t[:], EPS)
        b1_sb = cpool.tile([128, NJ], F32)
        nc.sync.dma_start(out=b1_sb[:], in_=b1t[:])
        g1b = cpool.tile([128, E], F32)
        nc.sync.dma_start(out=g1b[:], in_=_bcast(vg1[:], 128, E))
        be1b = cpool.tile([128, E], F32)
        nc.sync.dma_start(out=be1b[:], in_=_bcast(vbe1[:], 128, E))
        g2b = cpool.tile([128, E], F32)
        nc.sync.dma_start(out=g2b[:], in_=_bcast(vg2[:], 128, E))
        be2b = cpool.tile([128, E], F32)
        nc.sync.dma_start(out=be2b[:], in_=_bcast(vbe2[:], 128, E))
        bob = cpool.tile([128, E], F32)
        nc.sync.dma_start(out=bob[:], in_=_bcast(vbo[:], 128, E))
        b2b = cpool.tile([128, E], F32)
        nc.sync.dma_start(out=b2b[:], in_=_bcast(vb2[:], 128, E))

        with tc.tile_pool(name="attn", bufs=1) as apool:
            # projection weights (e-major 2d layout), bf16 [128, NE, 512]
            wq_sb = apool.tile([128, NE, E], BF16)
            wk_sb = apool.tile([128, NE, E], BF16)
            wv_sb = apool.tile([128, NE, E], BF16)
            wo_sb = apool.tile([128, NE, E], BF16)
            for e in range(NE):
                es = slice(e * 128, (e + 1) * 128)
                nc.gpsimd.dma_start(out=wq_sb[:, e, :], in_=wq[es, :])
                nc.gpsimd.dma_start(out=wk_sb[:, e, :], in_=wk[es, :])
                nc.gpsimd.dma_start(out=wv_sb[:, e, :], in_=wv[es, :])
                nc.gpsimd.dma_start(out=wo_sb[:, e, :], in_=wo[es, :])

            xn_mine = apool.tile([128, NTM, E], F32)
            kT_sb = apool.tile([128, NG, T], BF16)
            qT_sb = apool.tile([128, NG, LQ], BF16)
            vaug = apool.tile([128, NT, 8 * 65], BF16)
            oT_sb = apool.tile([128, NG, LQ], BF16)

            # ---- P1+P2a interleaved: LN1, transpose, K/V per 512-chunk ----
            with tc.tile_pool(name="pA", bufs=2) as pA:
                for c in range(4):
                    xnT_c = pA.tile([128, NE, 512], BF16, tag="xnT",
                                    name=f"xnT_c{c}")
                    for i in range(4):
                        it = c * 4 + i
                        x_t = wk_pool.tile([128, E], F32, tag="x_in")
                        nc.sync.dma_start(out=x_t[:],
                                          in_=xf[it * 128:(it + 1) * 128, :])
                        xnb = wk_pool.tile([128, E], BF16, tag="xnb")
                        _layernorm(nc, ln_pool, x_t[:], xnb[:], eps_t,
                                   g1b, be1b, tmp_pool=wk_pool)
                        for e in range(NE):
                            tp = ps.tile([128, 128], BF16, tag="mm")
                            nc.tensor.transpose(
                                tp[:], xnb[:, e * 128:(e + 1) * 128], ident[:])
                            nc.vector.tensor_copy(
                                out=xnT_c[:, e, i * 128:(i + 1) * 128],
                                in_=tp[:])
                    cs = slice(c * 512, (c + 1) * 512)
                    for g in range(NG):
                        kp = ps.tile([128, 512], F32, tag="mm")
                        for e in range(NE):
                            nc.tensor.matmul(
                                kp[:], wk_sb[:, e, g * 128:(g + 1) * 128],
                                xnT_c[:, e, :],
                                start=(e == 0), stop=(e == NE - 1))
                        nc.vector.tensor_copy(out=kT_sb[:, g, cs], in_=kp[:])
                    for i in range(4):
                        vp = ps.tile([128, 512], F32, tag="mm")
                        for e in range(NE):
                            nc.tensor.matmul(
                                vp[:], xnT_c[:, e, i * 128:(i + 1) * 128],
                                wv_sb[:, e, :], start=(e == 0),
                                stop=(e == NE - 1))
                        v3 = vaug[:, c * 4 + i, :].rearrange(
                            "p (h c) -> p h c", c=65)
                        nc.vector.tensor_copy(
                            out=v3[:, :, 0:64],
                            in_=vp[:].rearrange("p (h d) -> p h d", d=64))
                        nc.vector.memset(v3[:, :, 64], 1.0)

                # ---- P1b+P2b: LN1 own rows, transpose, Q per 512-chunk ----
                for qc in range(2):
                    xmT_c = pA.tile([128, NE, 512], BF16, tag="xnT",
                                    name=f"xmT_c{qc}")
                    for i in range(4):
                        im = qc * 4 + i
                        x_t = wk_pool.tile([128, E], F32, tag="x_in")
                        nc.sync.dma_start(out=x_t[:],
                                          in_=xm[im * 128:(im + 1) * 128, :])
                        _layernorm(nc, ln_pool, x_t[:], xn_mine[:, im, :],
                                   eps_t, g1b, be1b, tmp_pool=wk_pool)
                        xnb = wk_pool.tile([128, E], BF16, tag="xnb")
                        nc.vector.tensor_copy(out=xnb[:],
                                              in_=xn_mine[:, im, :])
                        for e in range(NE):
                            tp = ps.tile([128, 128], BF16, tag="mm")
                            nc.tensor.transpose(
                                tp[:], xnb[:, e * 128:(e + 1) * 128], ident[:])
                            nc.vector.tensor_copy(
                                out=xmT_c[:, e, i * 128:(i + 1) * 128],
                                in_=tp[:])
                    qs = slice(qc * 512, (qc + 1) * 512)
                    for g in range(NG):
                        qp = ps.tile([128, 512], F32, tag="mm")
                        for e in range(NE):
                            nc.tensor.matmul(
                                qp[:], wq_sb[:, e, g * 128:(g + 1) * 128],
                                xmT_c[:, e, :],
                                start=(e == 0), stop=(e == NE - 1))
                        nc.vector.tensor_copy(out=qT_sb[:, g, qs], in_=qp[:])

            # prefetch W1 during attention (W2 streams after the pool frees)
            w1_sb = pp.tile([128, NE, HID], BF16)
            for e in range(NE):
                nc.gpsimd.dma_start(out=w1_sb[:, e, :],
                                    in_=w1[e * 128:(e + 1) * 128, :])

            # ---- P3: attention per head ----
            ps_oT_cm = tc.tile_pool(name="ps_oT", bufs=2, space="PSUM")
            ps_oT = ps_oT_cm.__enter__()
            for h in range(H):
                g, r0 = h // 2, (h % 2) * 64
                oT_ps = ps_oT.tile([65, LQ], F32, tag="oT")
                for kb in range(NT):
                    c0 = kb * 64
                    chunks = [(c0, 512), (512, 1024)] if c0 < 512 else [(c0, 1024)]
                    for ci, (a, b_) in enumerate(chunks):
                        n = b_ - a
                        sc = ps.tile([128, 512], F32, tag="mm")
                        nc.tensor.matmul(
                            sc[:, :n],
                            kT_sb[r0:r0 + 64, g, kb * 128:(kb + 1) * 128],
                            qT_sb[r0:r0 + 64, g, a:b_],
                            start=True, stop=True)
                        ex = wk_pool.tile([128, 512], BF16, tag="exp")
                        nc.scalar.activation(out=ex[:, :n], in_=sc[:, :n],
                                             func=AF.Exp, scale=SCALE)
                        if ci == 0:
                            nc.vector.tensor_mul(ex[:, 0:64], ex[:, 0:64],
                                                 tri_sb[:])
                        nc.tensor.matmul(
                            oT_ps[:, a:b_],
                            vaug[:, kb, h * 65:(h + 1) * 65],
                            ex[:, :n],
                            start=(kb == 0), stop=(kb == NT - 1),
                            skip_group_check=True)
                se = wk_pool.tile([1, LQ], BF16, tag="se")
                nc.vector.reciprocal(out=se[:], in_=oT_ps[64:65, :])
                nc.vector.tensor_copy(out=oT_sb[r0:r0 + 64, g, :],
                                      in_=oT_ps[0:64, :])
                for c in range(2):
                    cs = slice(c * 512, (c + 1) * 512)
                    rb = ps.tile([64, 512], F32, tag="mm")
                    nc.tensor.matmul(rb[:], ones1[:], se[:, cs],
                                     start=True, stop=True)
                    nc.vector.tensor_mul(oT_sb[r0:r0 + 64, g, cs],
                                         oT_sb[r0:r0 + 64, g, cs], rb[:])

            ps_oT_cm.__exit__(None, None, None)

            # ---- P4: Wo + residual + LN2 + transpose ----
            xn2_sb = pp.tile([128, NTM, E], F32)
            xn2T = pp.tile([128, NE, LQ], BF16)
            for m in range(NTM):
                ms = slice(m * 128, (m + 1) * 128)
                mha = ps.tile([128, 512], F32, tag="mm")
                for g in range(NG):
                    nc.tensor.matmul(mha[:], oT_sb[:, g, ms], wo_sb[:, g, :],
                                     start=(g == 0), stop=(g == NG - 1))
                x2 = wk_pool.tile([128, E], F32, tag="x2")
                nc.vector.tensor_add(x2[:], mha[:], xn_mine[:, m, :])
                nc.vector.tensor_add(x2[:], x2[:], bob[:])
                _layernorm(nc, ln_pool, x2[:], xn2_sb[:, m, :], eps_t, g2b, be2b)
                xnb = wk_pool.tile([128, E], BF16, tag="xnb")
                nc.vector.tensor_copy(out=xnb[:], in_=xn2_sb[:, m, :])
                for e in range(NE):
                    tp = ps.tile([128, 128], BF16, tag="mm")
                    nc.tensor.transpose(tp[:], xnb[:, e * 128:(e + 1) * 128],
                                        ident[:])
                    nc.vector.tensor_copy(
                        out=xn2T[:, e, m * 128:(m + 1) * 128], in_=tp[:])

        # ---- P5: FFN (attention pool freed; W1/W2 held in bf16) ----
        with tc.tile_pool(name="ffn", bufs=1) as fpool:
            w1_sb = fpool.tile([128, NE, HID], BF16)
            for e in range(NE):
                nc.gpsimd.dma_start(out=w1_sb[:, e, :],
                                    in_=w1[e * 128:(e + 1) * 128, :])
            ps_y_cm = tc.tile_pool(name="ps_y", bufs=1, space="PSUM")
            ps_y = ps_y_cm.__enter__()
            w2_sb = fpool.tile([128, NJ, E], BF16)
            nc.gpsimd.dma_start(
                out=w2_sb[:],
                in_=w2.rearrange("(j p) e -> p j e", p=128))
            for c in range(2):
                cs = slice(c * 512, (c + 1) * 512)
                y_ps = [ps_y.tile([128, 512], F32, tag=f"y{tt}",
                                  name=f"y_ps{tt}")
                        for tt in range(4)]
                for j in range(NJ):
                    hp = ps.tile([128, 512], F32, tag="mm")
                    for e in range(NE):
                        nc.tensor.matmul(
                            hp[:], w1_sb[:, e, j * 128:(j + 1) * 128],
                            xn2T[:, e, cs], start=(e == 0), stop=(e == NE - 1))
                    hr = wk_pool.tile([128, 512], BF16, tag="hr")
                    nc.scalar.activation(out=hr[:], in_=hp[:], func=AF.Relu,
                                         bias=b1_sb[:, j:j + 1], scale=1.0)
                    for tt in range(4):
                        nc.tensor.matmul(y_ps[tt][:],
                                         hr[:, tt * 128:(tt + 1) * 128],
                                         w2_sb[:, j, :],
                                         start=(j == 0), stop=(j == NJ - 1))
                for tt in range(4):
                    m = c * 4 + tt
                    yo = wk_pool.tile([128, E], F32, tag="yo")
                    nc.vector.tensor_add(yo[:], y_ps[tt][:], xn2_sb[:, m, :])
                    nc.vector.tensor_add(yo[:], yo[:], b2b[:])
                    nc.sync.dma_start(out=out[m * 128:(m + 1) * 128, :],
                                      in_=yo[:])
            ps_y_cm.__exit__(None, None, None)
    nc.compile()
    return nc


_prog_cache = {}


def _get_program():
    if "nc" not in _prog_cache:
        _prog_cache["nc"] = build_program()
    return _prog_cache["nc"]


def make_in_maps(x, Wq, Wk, Wv, Wo, bo, W1, b1, W2, b2, g1, be1, g2, be2):
    f = lambda a: np.ascontiguousarray(np.asarray(a, dtype=np.float32))
    x = f(x)
    shared = {
        "wq2d": f(np.transpose(np.asarray(Wq), (1, 0, 2)).reshape(E, E)),
        "wk2d": f(np.transpose(np.asarray(Wk), (1, 0, 2)).reshape(E, E)),
        "wv2d": f(np.transpose(np.asarray(Wv), (1, 0, 2)).reshape(E, E)),
        "wo": f(Wo), "w1": f(W1), "w2": f(W2),
        "b1t": f(np.asarray(b1).reshape(NJ, 128).T),
        "g1": f(g1), "be1": f(be1), "g2": f(g2), "be2": f(be2),
        "bo": f(bo), "b2": f(b2),
    }
    in_maps = []
    s_idx = np.arange(128)[:, None]
    p_idx = np.arange(64)[None, :]
    for c in range(8):
        b, h = c // 2, c % 2
        m = dict(shared)
        m["x_full"] = x[b]
        m["x_mine"] = f(x[b].reshape(NT, 2, 64, E)[:, h].reshape(T // 2, E))
        m["tri"] = f((s_idx <= h * 64 + p_idx).astype(np.float32))
        in_maps.append(m)
    return in_maps


def assemble(results):
    out = np.empty((B, T, E), dtype=np.float32)
    for c in range(8):
        b, h = c // 2, c % 2
        out[b].reshape(NT, 2, 64, E)[:, h] = \
            results[c]["out"].reshape(NT, 64, E)
    return out


def kernel(**inputs):
    nc = _get_program()
    in_maps = make_in_maps(**inputs)
    res = run_bass_kernel_spmd(nc, in_maps, core_ids=list(range(8)))
    return assemble(res.results)
